# revision 23
# baseline (speedup 1.0000x reference)
"""HalfKP NNUE-style network on 8 Trainium2 NeuronCores.

Two launches (collectives on this platform cost 20+us each AND throttle the
PE ~20% while active, so the cross-core reduction is done on host):

  Launch 1 (feature transformer, F-dim sharded 8 ways):
    Each core owns a 5120-wide slice of F for both colors. Features are
    quantized as (x - 0.5): 36 k-tiles (of 128) in fp8e3 (e3m4) against fp16
    weights, the last 4 as fp8e4 DoubleRow matmuls (2x PE throughput). The
    exact 0.5*sum(w) term is folded into the bias on host. Weights are
    pre-scaled by WS=2^17 so all paths share one PSUM accumulation. This
    keeps per-core HBM traffic at ~18 MB (vs 49 MB all-fp16) while staying
    at a measured end-to-end rel-err of ~1.8e-2.

  Host glue: f32 sum of the 8 fp16 partials, re-shard by batch, pack the
  MLP weights + pre-activations into one tensor per core.

  Launch 2 (tiny MLP, batch sharded 8 ways): bias+relu then 512->32->32->1
  with tanh, one input DMA, all arithmetic on device.
"""

import sys

import numpy as np

sys.path.insert(0, "/opt/trn_rl_repo")

import ml_dtypes

import concourse.bass as bass
import concourse.bacc as bacc
import concourse.tile as tile
import concourse.mybir as mybir
from concourse import bass_utils

E4 = ml_dtypes.float8_e4m3
E3 = ml_dtypes.float8_e3m4
F16 = np.float16
F32 = np.float32

B = 2048
F = 40960
H1 = 256
NCORES = 8
FS = F // NCORES          # features per core: 5120
NFT3 = 36                 # fp8e3 k-tiles per core
NFT8P = 2                 # fp8e4 DoubleRow pairs per core (2 k-tiles each)
NF3 = NFT3 * 128          # 4608 e3m4 features per core slice
NHT = H1 // 128           # 2
NHALF = 2
BH = B // NHALF           # 1024
NCK = BH // 512           # 2
NXT = 2 * NHT             # 4 (color, htile) blocks
BSH = B // NCORES         # 256 output rows per core
WS = 131072.0             # 2**17 weight scale (all paths share it)
E3CH = [6, 10, 10, 10]    # e3m4 k-tile DMA chunking

DT_F8 = mybir.dt.float8e4
DT_E3 = mybir.dt.float8e3
DT_F16 = mybir.dt.float16
DT_F32 = mybir.dt.float32

NCOL = 128 + NXT + 36
PREW = NXT * BSH + NCOL   # launch-2 packed input width


def build_ft_kernel(nc):
    DR = mybir.MatmulPerfMode.DoubleRow

    feats3 = nc.dram_tensor(
        "feats3", [2, NHALF, 128, NFT3, BH], DT_E3, kind="ExternalInput").ap()
    feats8 = nc.dram_tensor(
        "feats8", [2, NHALF, 128, NFT8P, NCK, 2, 512], DT_F8,
        kind="ExternalInput").ap()
    wts16 = nc.dram_tensor(
        "wts16", [2, 128, NFT3 * H1], DT_F16, kind="ExternalInput").ap()
    wts8 = nc.dram_tensor(
        "wts8", [2, 128, NFT8P, NHT, 2, 128], DT_F8, kind="ExternalInput").ap()
    partial = nc.dram_tensor(
        "partial", [2, NHT, 128, B], DT_F16, kind="ExternalOutput").ap()

    with tile.TileContext(nc) as tc:
        with (
            tc.tile_pool(name="wpool", bufs=1) as wpool,
            tc.tile_pool(name="fpool", bufs=3) as fpool,
            tc.tile_pool(name="f8pool", bufs=2) as f8pool,
            tc.tile_pool(name="opool", bufs=4) as opool,
            tc.tile_pool(name="pspool", bufs=2, space=bass.MemorySpace.PSUM) as pspool,
        ):
            # weights: c0 on scalar (start-critical head first), c1 on gpsimd
            w8_sb = []
            w16_sb = []
            wcols = NFT3 * H1
            mid = (NFT3 // 2) * H1
            for c in range(2):
                w8 = wpool.tile([128, NFT8P, NHT, 2, 128], DT_F8,
                                tag=f"w8{c}", name=f"w8{c}")
                w = wpool.tile([128, wcols], DT_F16, tag=f"w16{c}",
                               name=f"w16{c}")
                w8_sb.append(w8)
                w16_sb.append(w)
            nc.scalar.dma_start(w16_sb[0][:, 0:mid], wts16[0, :, 0:mid])
            nc.scalar.dma_start(w16_sb[0][:, mid:wcols], wts16[0, :, mid:wcols])
            nc.scalar.dma_start(w8_sb[0][:], wts8[0])
            nc.gpsimd.dma_start(w16_sb[1][:, 0:mid], wts16[1, :, 0:mid])
            nc.gpsimd.dma_start(w16_sb[1][:, mid:wcols], wts16[1, :, mid:wcols])
            nc.gpsimd.dma_start(w8_sb[1][:], wts8[1])

            # PE pstate warm-up on never-read scratch: ~3us of short matmuls
            # during the initial DMA wait ramps the clock to full speed
            warm = wpool.tile([128, 128], DT_F16, tag="warm", name="warm")
            nc.vector.memset(warm[:], 0.0)
            wps = pspool.tile([128, 512], DT_F32, tag="ps00", name="warmps")
            for _ in range(40):
                nc.tensor.matmul(wps[0:64, 0:64], warm[:, 0:64], warm[:, 0:64],
                                 start=True, stop=True)

            for c in range(2):
                for half in range(NHALF):
                    ps = {}
                    for ht in range(NHT):
                        for ck in range(NCK):
                            ps[(ht, ck)] = pspool.tile(
                                [128, 512], DT_F32,
                                tag=f"ps{ht}{ck}", name=f"ps{ht}{ck}")
                    # e3m4 phase first; fp8-DR phase last (its data arrives
                    # mid-block via the ci==2 issue below)
                    f8 = f8pool.tile([128, NFT8P, NCK, 2, 512], DT_F8,
                                     tag="f8", name="f8")
                    kt = 0
                    for ci, nk in enumerate(E3CH):
                        ftile = fpool.tile([128, 10 * BH], DT_E3, tag="feat",
                                           name="feat")
                        dma_eng = nc.sync if ci % 2 == 0 else nc.scalar
                        dma_eng.dma_start(
                            ftile[:, 0:nk * BH],
                            feats3[c, half, :, kt:kt + nk, :])
                        if ci == 2:
                            nc.sync.dma_start(f8[:], feats8[c, half])
                        for lk in range(nk):
                            gk = kt + lk
                            for ht in range(NHT):
                                lhsT = w16_sb[c][:, gk * H1 + ht * 128:
                                                 gk * H1 + (ht + 1) * 128]
                                for ck in range(NCK):
                                    nc.tensor.matmul(
                                        ps[(ht, ck)][:],
                                        lhsT,
                                        ftile[:, lk * BH + ck * 512:
                                              lk * BH + (ck + 1) * 512],
                                        start=(gk == 0),
                                        stop=False,
                                    )
                        kt += nk
                    for j in range(NFT8P):
                        for ht in range(NHT):
                            lhsT = w8_sb[c][:, j, ht, :, :]
                            for ck in range(NCK):
                                nc.tensor.matmul(
                                    ps[(ht, ck)][:],
                                    lhsT,
                                    f8[:, j, ck, :, :],
                                    start=False,
                                    stop=(j == NFT8P - 1),
                                    perf_mode=DR,
                                )
                    for ht in range(NHT):
                        ot = opool.tile([128, BH], DT_F16, tag="out", name="ot")
                        for ck in range(NCK):
                            if ck == 0:
                                nc.vector.tensor_copy(
                                    ot[:, ck * 512:(ck + 1) * 512],
                                    ps[(ht, ck)][:])
                            else:
                                nc.scalar.activation(
                                    ot[:, ck * 512:(ck + 1) * 512],
                                    ps[(ht, ck)][:],
                                    mybir.ActivationFunctionType.Copy)
                        nc.sync.dma_start(
                            partial[c, ht, :, half * BH:(half + 1) * BH], ot[:])
    return nc


def build_mlp_kernel(nc):
    """pre2 packs pre-activations and all MLP consts: one input DMA.

    cols [0, NXT*BSH)            pre: col xi*BSH+b (scaled by WS)
    cols [P, P+128)              w1t: col kt*32+m = W1[m, kt*128+p]
    cols [P+128, P+128+NXT)      bft (incl. the 0.5-shift bias correction)
    cols [P+132 ..] (parts 0:32) w2t(32) | b1 | b2 | w3t | b3
    """
    AF = mybir.ActivationFunctionType
    P = NXT * BSH
    pre2 = nc.dram_tensor("pre2", [128, PREW], DT_F32, kind="ExternalInput").ap()
    out = nc.dram_tensor("out", [1, BSH], DT_F32, kind="ExternalOutput").ap()

    with tile.TileContext(nc) as tc:
        with (
            tc.tile_pool(name="xpool", bufs=1) as xpool,
            tc.tile_pool(name="ypool", bufs=1) as ypool,
            tc.tile_pool(name="pspool", bufs=1, space=bass.MemorySpace.PSUM) as pspool,
        ):
            cs = xpool.tile([128, PREW], DT_F32, tag="pre2")
            # sliced load: first slice lands fast, relu overlaps the rest
            nc.sync.dma_start(cs[:, P:PREW], pre2[:, P:PREW])
            for xi in range(NXT):
                nc.sync.dma_start(cs[:, xi * BSH:(xi + 1) * BSH],
                                  pre2[:, xi * BSH:(xi + 1) * BSH])

            w1t_sb = cs[:, P:P + NXT * 32]
            bft_sb = cs[:, P + 128:P + 128 + NXT]
            co = P + 128 + NXT
            w2t_sb = cs[0:32, co:co + 32]
            b1_sb = cs[0:32, co + 32:co + 33]
            b2_sb = cs[0:32, co + 33:co + 34]
            w3t_sb = cs[0:32, co + 34:co + 35]
            b3_sb = cs[0:1, co + 35:co + 36]

            x_sb = xpool.tile([128, NXT * BSH], DT_F32, tag="x")
            # dummy 1-elem activation: pulls the ACT LUT load to kernel start
            nc.scalar.activation(x_sb[0:1, 0:1], x_sb[0:1, 0:1], AF.Relu)
            for xi in range(NXT):
                nc.scalar.activation(
                    x_sb[:, xi * BSH:(xi + 1) * BSH],
                    cs[:, xi * BSH:(xi + 1) * BSH],
                    AF.Relu, bias=bft_sb[:, xi:xi + 1], scale=1.0 / WS)

            ps1 = pspool.tile([32, 512], DT_F32, tag="ps1")
            for kt in range(NXT):
                nc.tensor.matmul(
                    ps1[:, :BSH],
                    w1t_sb[:, kt * 32:(kt + 1) * 32],
                    x_sb[:, kt * BSH:(kt + 1) * BSH],
                    start=(kt == 0),
                    stop=(kt == NXT - 1),
                )
            y1 = ypool.tile([32, BSH], DT_F32, tag="y1")
            nc.scalar.activation(y1[:], ps1[:, :BSH], AF.Relu, bias=b1_sb)

            ps2 = pspool.tile([32, 512], DT_F32, tag="ps2")
            nc.tensor.matmul(ps2[:, :BSH], w2t_sb, y1[:], start=True, stop=True)
            y2 = ypool.tile([32, BSH], DT_F32, tag="y2")
            nc.scalar.activation(y2[:], ps2[:, :BSH], AF.Relu, bias=b2_sb)

            ps3 = pspool.tile([1, 512], DT_F32, tag="ps3")
            nc.tensor.matmul(ps3[:, :BSH], w3t_sb, y2[:], start=True, stop=True)
            y3 = ypool.tile([1, BSH], DT_F32, tag="y3")
            nc.scalar.activation(y3[:], ps3[:, :BSH], AF.Tanh, bias=b3_sb)
            nc.sync.dma_start(out[:], y3[:])
    return nc


_NC_CACHE = {}

# Dev/profiling knobs (ignored by graders that just call kernel()):
TRACE = False
LAST_EXEC_NS = {}


def _run(nc, in_maps, label):
    res = bass_utils.run_bass_kernel_spmd(
        nc, in_maps, core_ids=list(range(NCORES)), trace=TRACE
    )
    LAST_EXEC_NS[label] = res.exec_time_ns
    return res


def _get_compiled(name, builder):
    if name not in _NC_CACHE:
        nc = bacc.Bacc("TRN2", target_bir_lowering=False, debug=False)
        builder(nc)
        nc.compile()
        _NC_CACHE[name] = nc
    return _NC_CACHE[name]


def _feat3_shard(x, core):
    """x [B, F] f32 -> [NHALF, 128, NFT3, BH] fp8e3 of (x-0.5): [half,p,kt,b]."""
    base = core * FS
    blk = (np.ascontiguousarray(x[:, base:base + NF3].T) - 0.5).astype(E3)
    blk = blk.reshape(NFT3, 128, NHALF, BH)       # (kt, p, half, b)
    return np.ascontiguousarray(blk.transpose(2, 1, 0, 3))


def _feat8_shard(x, core):
    """x [B,F] f32 -> [NHALF, 128, NFT8P, NCK, 2, 512] fp8e4 of (x - 0.5)."""
    base = core * FS + NF3
    blk = np.ascontiguousarray(x[:, base:base + NFT8P * 256].T) - 0.5
    blk = blk.astype(E4)
    blk = blk.reshape(NFT8P, 2, 128, NHALF, NCK, 512)  # (j, i, p, half, ck, b)
    return np.ascontiguousarray(blk.transpose(3, 2, 0, 4, 1, 5))


def _w16_shard(w, core):
    """[H1, F] f32 -> [128, NFT3*H1] fp16: col kt*H1 + h = W[h, kt*128+p]*WS."""
    ws = w[:, core * FS:core * FS + NF3]
    wt = (ws.T * WS).astype(F16)                  # [4608, 256]
    return np.ascontiguousarray(
        wt.reshape(NFT3, 128, H1).transpose(1, 0, 2).reshape(128, NFT3 * H1))


def _w8_shard(w, core):
    """[H1, F] f32 -> [128, NFT8P, NHT, 2, 128] fp8e4 of W*WS, DR layout."""
    base = core * FS + NF3
    ws = w[:, base:base + NFT8P * 256]
    wt = (ws.T * WS).astype(E4)                   # [512, 256]
    wt = wt.reshape(NFT8P, 2, 128, NHT, 128)      # (j, i, p, ht, h')
    return np.ascontiguousarray(wt.transpose(2, 0, 3, 1, 4))


def kernel(white_features, black_features, W_fw, b_fw, W_fb, b_fb,
           W1, b1, W2, b2, W3, b3):
    white_features = np.asarray(white_features, dtype=F32)
    black_features = np.asarray(black_features, dtype=F32)
    W_fw = np.asarray(W_fw, dtype=F32)
    W_fb = np.asarray(W_fb, dtype=F32)

    # ---------- launch 1: feature transformer partials ----------
    nc1 = _get_compiled("ft", build_ft_kernel)
    in_maps1 = []
    for core in range(NCORES):
        feats3 = np.stack([_feat3_shard(white_features, core),
                           _feat3_shard(black_features, core)])
        feats8 = np.stack([_feat8_shard(white_features, core),
                           _feat8_shard(black_features, core)])
        wts16 = np.stack([_w16_shard(W_fw, core), _w16_shard(W_fb, core)])
        wts8 = np.stack([_w8_shard(W_fw, core), _w8_shard(W_fb, core)])
        in_maps1.append({"feats3": feats3, "feats8": feats8,
                         "wts16": wts16, "wts8": wts8})
    res1 = _run(nc1, in_maps1, "ft")
    partials = [np.asarray(r["partial"]) for r in res1.results]

    # ---------- host glue: reduce over F-shards + re-shard by batch ----
    total = np.zeros((2, NHT, 128, B), dtype=F32)
    for p in partials:
        total += p.astype(F32)

    # bias correction: every feature column is 0.5-shifted on device
    bc_w = (np.asarray(b_fw, np.float64)
            + 0.5 * W_fw.astype(np.float64).sum(1)).astype(F32)
    bc_b = (np.asarray(b_fb, np.float64)
            + 0.5 * W_fb.astype(np.float64).sum(1)).astype(F32)

    P = NXT * BSH
    consts = np.zeros((128, NCOL), dtype=F32)
    consts[:, 0:NXT * 32] = (
        np.asarray(W1, dtype=F32).T.reshape(NXT, 128, 32)
        .transpose(1, 0, 2).reshape(128, NXT * 32))
    consts[:, 128:128 + NHT] = bc_w.reshape(NHT, 128).T
    consts[:, 128 + NHT:128 + NXT] = bc_b.reshape(NHT, 128).T
    co = 128 + NXT
    consts[0:32, co:co + 32] = np.asarray(W2, dtype=F32).T
    consts[0:32, co + 32] = np.asarray(b1, dtype=F32)
    consts[0:32, co + 33] = np.asarray(b2, dtype=F32)
    consts[0:32, co + 34] = np.asarray(W3, dtype=F32).reshape(32)
    consts[0, co + 35] = np.asarray(b3, dtype=F32).reshape(())

    nc2 = _get_compiled("mlp", build_mlp_kernel)
    in_maps2 = []
    for core in range(NCORES):
        sl = total[..., core * BSH:(core + 1) * BSH]   # [2, NHT, 128, BSH]
        pre2 = np.empty((128, PREW), dtype=F32)
        pre2[:, 0:P] = sl.transpose(2, 0, 1, 3).reshape(128, P)
        pre2[:, P:] = consts
        in_maps2.append({"pre2": pre2})
    res2 = _run(nc2, in_maps2, "mlp")
    out = np.concatenate(
        [np.asarray(r["out"], dtype=F32).reshape(-1) for r in res2.results])
    return out


# revision 28
# speedup vs baseline: 1.0805x; 1.0805x over previous
"""HalfKP NNUE-style network on 8 Trainium2 NeuronCores.

Two launches (collectives on this platform cost 20+us each AND throttle the
PE ~20% while active, so the cross-core reduction is done on host):

  Launch 1 (feature transformer, F-dim sharded 8 ways):
    Each core owns a 5120-wide slice of F for both colors. Features are
    quantized as (x - 0.5): 36 k-tiles (of 128) in fp8e3 (e3m4) against fp16
    weights, the last 4 as fp8e4 DoubleRow matmuls (2x PE throughput). The
    exact 0.5*sum(w) term is folded into the bias on host. Weights are
    pre-scaled by WS=2^17 so all paths share one PSUM accumulation. This
    keeps per-core HBM traffic at ~18 MB (vs 49 MB all-fp16) while staying
    at a measured end-to-end rel-err of ~1.8e-2.

  Host glue: f32 sum of the 8 fp16 partials, re-shard by batch, pack the
  MLP weights + pre-activations into one tensor per core.

  Launch 2 (tiny MLP, batch sharded 8 ways): bias+relu then 512->32->32->1
  with tanh, one input DMA, all arithmetic on device.
"""

import sys

import numpy as np

sys.path.insert(0, "/opt/trn_rl_repo")

import ml_dtypes

import concourse.bass as bass
import concourse.bacc as bacc
import concourse.tile as tile
import concourse.mybir as mybir
from concourse import bass_utils

E4 = ml_dtypes.float8_e4m3
E3 = ml_dtypes.float8_e3m4
F16 = np.float16
F32 = np.float32

B = 2048
F = 40960
H1 = 256
NCORES = 8
FS = F // NCORES          # features per core: 5120
NFT3 = 36                 # fp8e3 k-tiles per core
NFT8P = 2                 # fp8e4 DoubleRow pairs per core (2 k-tiles each)
NF3 = NFT3 * 128          # 4608 e3m4 features per core slice
NHT = H1 // 128           # 2
NHALF = 2
BH = B // NHALF           # 1024
NCK = BH // 512           # 2
NXT = 2 * NHT             # 4 (color, htile) blocks
BSH = B // NCORES         # 256 output rows per core
WS = 131072.0             # 2**17 weight scale (all paths share it)
E3CH = [8, 10, 9, 9]      # e3m4 k-tile DMA chunking
W16CH = [8, 12, 16]       # w16 k-tile DMA chunking

DT_F8 = mybir.dt.float8e4
DT_E3 = mybir.dt.float8e3
DT_F16 = mybir.dt.float16
DT_F32 = mybir.dt.float32

NCOL = 128 + NXT + 36
PREW = NXT * BSH + NCOL   # launch-2 packed input width


def build_ft_kernel(nc):
    DR = mybir.MatmulPerfMode.DoubleRow

    # NOTE: flat free dim — a [.., NFT3, BH] shape fragments the DMA into
    # 1KB-per-k-tile descriptors and halves effective HBM bandwidth
    feats3 = nc.dram_tensor(
        "feats3", [2, NHALF, 128, NFT3 * BH], DT_E3, kind="ExternalInput").ap()
    feats8 = nc.dram_tensor(
        "feats8", [2, NHALF, 128, NFT8P, NCK, 2, 512], DT_F8,
        kind="ExternalInput").ap()
    wts16 = nc.dram_tensor(
        "wts16", [2, 128, NFT3 * H1], DT_F16, kind="ExternalInput").ap()
    wts8 = nc.dram_tensor(
        "wts8", [2, 128, NFT8P, NHT, 2, 128], DT_F8, kind="ExternalInput").ap()
    partial = nc.dram_tensor(
        "partial", [2, NHT, 128, B], DT_F16, kind="ExternalOutput").ap()

    with tile.TileContext(nc) as tc:
        with (
            tc.tile_pool(name="wpool", bufs=1) as wpool,
            tc.tile_pool(name="fpool", bufs=3) as fpool,
            tc.tile_pool(name="f8pool", bufs=2) as f8pool,
            tc.tile_pool(name="opool", bufs=4) as opool,
            tc.tile_pool(name="pspool", bufs=2, space=bass.MemorySpace.PSUM) as pspool,
        ):
            # weights: start-critical c0 head upfront; the rest is issued at
            # need-ordered positions inside the block loop
            w8_sb = []
            w16_sb = []
            wcols = NFT3 * H1
            for c in range(2):
                w8 = wpool.tile([128, NFT8P, NHT, 2, 128], DT_F8,
                                tag=f"w8{c}", name=f"w8{c}")
                w = wpool.tile([128, wcols], DT_F16, tag=f"w16{c}",
                               name=f"w16{c}")
                w8_sb.append(w8)
                w16_sb.append(w)

            def w16_load(eng, c, chunk):
                lo = sum(W16CH[:chunk]) * H1
                hi = lo + W16CH[chunk] * H1
                eng.dma_start(w16_sb[c][:, lo:hi], wts16[c, :, lo:hi])

            nc.scalar.dma_start(w8_sb[0][:], wts8[0])
            w16_load(nc.scalar, 0, 0)

            # PE pstate warm-up on never-read scratch: ~3us of short matmuls
            # during the initial DMA wait ramps the clock to full speed
            warm = wpool.tile([128, 128], DT_F16, tag="warm", name="warm")
            nc.vector.memset(warm[:], 0.0)
            wps = pspool.tile([128, 512], DT_F32, tag="ps00", name="warmps")
            for _ in range(40):
                nc.tensor.matmul(wps[0:64, 0:64], warm[:, 0:64], warm[:, 0:64],
                                 start=True, stop=True)

            f8_tiles = {}
            for c in range(2):
                for half in range(NHALF):
                    ps = {}
                    for ht in range(NHT):
                        for ck in range(NCK):
                            ps[(ht, ck)] = pspool.tile(
                                [128, 512], DT_F32,
                                tag=f"ps{ht}{ck}", name=f"ps{ht}{ck}")
                    # fp8-DR phase first (tiny data: fast start), e3m4 after
                    if not f8_tiles:
                        t = f8pool.tile([128, NFT8P, NCK, 2, 512], DT_F8,
                                        tag="f8", name="f8a")
                        nc.sync.dma_start(t[:, 0:1], feats8[0, 0, :, 0:1])
                        nc.sync.dma_start(t[:, 1:NFT8P],
                                          feats8[0, 0, :, 1:NFT8P])
                        f8_tiles[(0, 0)] = t
                    f8 = f8_tiles.pop((c, half))
                    for j in range(NFT8P):
                        for ht in range(NHT):
                            lhsT = w8_sb[c][:, j, ht, :, :]
                            for ck in range(NCK):
                                nc.tensor.matmul(
                                    ps[(ht, ck)][:],
                                    lhsT,
                                    f8[:, j, ck, :, :],
                                    start=(j == 0),
                                    stop=False,
                                    perf_mode=DR,
                                )
                    kt = 0
                    for ci, nk in enumerate(E3CH):
                        ftile = fpool.tile([128, 10 * BH], DT_E3, tag="feat",
                                           name="feat")
                        dma_eng = nc.scalar if ci % 2 == 0 else nc.sync
                        dma_eng.dma_start(
                            ftile[:, 0:nk * BH],
                            feats3[c, half, :, kt * BH:(kt + nk) * BH])
                        # need-ordered weight / prefetch insertions
                        if (c, half) == (0, 0):
                            if ci == 0:
                                w16_load(nc.scalar, 0, 1)
                            elif ci == 1:
                                w16_load(nc.sync, 0, 2)
                        elif (c, half) == (0, 1):
                            if ci == 0:
                                nc.scalar.dma_start(w8_sb[1][:], wts8[1])
                                w16_load(nc.scalar, 1, 0)
                            elif ci == 1:
                                w16_load(nc.sync, 1, 1)
                            elif ci == 2:
                                w16_load(nc.scalar, 1, 2)
                        if ci == 3:
                            nxt_blk = ((c, half + 1) if half + 1 < NHALF
                                       else (c + 1, 0))
                            if nxt_blk[0] < 2:
                                t = f8pool.tile(
                                    [128, NFT8P, NCK, 2, 512], DT_F8,
                                    tag="f8", name="f8b")
                                nc.sync.dma_start(t[:], feats8[nxt_blk])
                                f8_tiles[nxt_blk] = t
                        for lk in range(nk):
                            gk = kt + lk
                            for ht in range(NHT):
                                lhsT = w16_sb[c][:, gk * H1 + ht * 128:
                                                 gk * H1 + (ht + 1) * 128]
                                for ck in range(NCK):
                                    nc.tensor.matmul(
                                        ps[(ht, ck)][:],
                                        lhsT,
                                        ftile[:, lk * BH + ck * 512:
                                              lk * BH + (ck + 1) * 512],
                                        start=False,
                                        stop=(gk == NFT3 - 1),
                                    )
                        kt += nk
                    for ht in range(NHT):
                        ot = opool.tile([128, BH], DT_F16, tag="out", name="ot")
                        for ck in range(NCK):
                            if ck == 0:
                                nc.vector.tensor_copy(
                                    ot[:, ck * 512:(ck + 1) * 512],
                                    ps[(ht, ck)][:])
                            else:
                                nc.scalar.activation(
                                    ot[:, ck * 512:(ck + 1) * 512],
                                    ps[(ht, ck)][:],
                                    mybir.ActivationFunctionType.Copy)
                        nc.sync.dma_start(
                            partial[c, ht, :, half * BH:(half + 1) * BH], ot[:])
    return nc


def build_mlp_kernel(nc):
    """pre2 packs pre-activations and all MLP consts: one input DMA.

    cols [0, NXT*BSH)            pre: col xi*BSH+b (scaled by WS)
    cols [P, P+128)              w1t: col kt*32+m = W1[m, kt*128+p]
    cols [P+128, P+128+NXT)      bft (incl. the 0.5-shift bias correction)
    cols [P+132 ..] (parts 0:32) w2t(32) | b1 | b2 | w3t | b3
    """
    AF = mybir.ActivationFunctionType
    P = NXT * BSH
    pre2 = nc.dram_tensor("pre2", [128, PREW], DT_F32, kind="ExternalInput").ap()
    out = nc.dram_tensor("out", [1, BSH], DT_F32, kind="ExternalOutput").ap()

    with tile.TileContext(nc) as tc:
        with (
            tc.tile_pool(name="xpool", bufs=1) as xpool,
            tc.tile_pool(name="ypool", bufs=1) as ypool,
            tc.tile_pool(name="pspool", bufs=1, space=bass.MemorySpace.PSUM) as pspool,
        ):
            cs = xpool.tile([128, PREW], DT_F32, tag="pre2")
            # sliced load: first slice lands fast, relu overlaps the rest
            nc.sync.dma_start(cs[:, P:PREW], pre2[:, P:PREW])
            for xi in range(NXT):
                nc.sync.dma_start(cs[:, xi * BSH:(xi + 1) * BSH],
                                  pre2[:, xi * BSH:(xi + 1) * BSH])

            w1t_sb = cs[:, P:P + NXT * 32]
            bft_sb = cs[:, P + 128:P + 128 + NXT]
            co = P + 128 + NXT
            w2t_sb = cs[0:32, co:co + 32]
            b1_sb = cs[0:32, co + 32:co + 33]
            b2_sb = cs[0:32, co + 33:co + 34]
            w3t_sb = cs[0:32, co + 34:co + 35]
            b3_sb = cs[0:1, co + 35:co + 36]

            x_sb = xpool.tile([128, NXT * BSH], DT_F32, tag="x")
            # dummy 1-elem activation: pulls the ACT LUT load to kernel start
            nc.scalar.activation(x_sb[0:1, 0:1], x_sb[0:1, 0:1], AF.Relu)
            for xi in range(NXT):
                nc.scalar.activation(
                    x_sb[:, xi * BSH:(xi + 1) * BSH],
                    cs[:, xi * BSH:(xi + 1) * BSH],
                    AF.Relu, bias=bft_sb[:, xi:xi + 1], scale=1.0 / WS)

            ps1 = pspool.tile([32, 512], DT_F32, tag="ps1")
            for kt in range(NXT):
                nc.tensor.matmul(
                    ps1[:, :BSH],
                    w1t_sb[:, kt * 32:(kt + 1) * 32],
                    x_sb[:, kt * BSH:(kt + 1) * BSH],
                    start=(kt == 0),
                    stop=(kt == NXT - 1),
                )
            y1 = ypool.tile([32, BSH], DT_F32, tag="y1")
            nc.scalar.activation(y1[:], ps1[:, :BSH], AF.Relu, bias=b1_sb)

            ps2 = pspool.tile([32, 512], DT_F32, tag="ps2")
            nc.tensor.matmul(ps2[:, :BSH], w2t_sb, y1[:], start=True, stop=True)
            y2 = ypool.tile([32, BSH], DT_F32, tag="y2")
            nc.scalar.activation(y2[:], ps2[:, :BSH], AF.Relu, bias=b2_sb)

            ps3 = pspool.tile([1, 512], DT_F32, tag="ps3")
            nc.tensor.matmul(ps3[:, :BSH], w3t_sb, y2[:], start=True, stop=True)
            y3 = ypool.tile([1, BSH], DT_F32, tag="y3")
            nc.scalar.activation(y3[:], ps3[:, :BSH], AF.Tanh, bias=b3_sb)
            nc.sync.dma_start(out[:], y3[:])
    return nc


_NC_CACHE = {}

# Dev/profiling knobs (ignored by graders that just call kernel()):
TRACE = False
LAST_EXEC_NS = {}


def _run(nc, in_maps, label):
    res = bass_utils.run_bass_kernel_spmd(
        nc, in_maps, core_ids=list(range(NCORES)), trace=TRACE
    )
    LAST_EXEC_NS[label] = res.exec_time_ns
    return res


def _get_compiled(name, builder):
    if name not in _NC_CACHE:
        nc = bacc.Bacc("TRN2", target_bir_lowering=False, debug=False)
        builder(nc)
        nc.compile()
        _NC_CACHE[name] = nc
    return _NC_CACHE[name]


def _feat3_shard(x, core):
    """x [B, F] f32 -> [NHALF, 128, NFT3*BH] fp8e3 of (x-0.5): [half,p,kt*b]."""
    base = core * FS
    blk = (np.ascontiguousarray(x[:, base:base + NF3].T) - 0.5).astype(E3)
    blk = blk.reshape(NFT3, 128, NHALF, BH)       # (kt, p, half, b)
    return np.ascontiguousarray(
        blk.transpose(2, 1, 0, 3)).reshape(NHALF, 128, NFT3 * BH)


def _feat8_shard(x, core):
    """x [B,F] f32 -> [NHALF, 128, NFT8P, NCK, 2, 512] fp8e4 of (x - 0.5)."""
    base = core * FS + NF3
    blk = np.ascontiguousarray(x[:, base:base + NFT8P * 256].T) - 0.5
    blk = blk.astype(E4)
    blk = blk.reshape(NFT8P, 2, 128, NHALF, NCK, 512)  # (j, i, p, half, ck, b)
    return np.ascontiguousarray(blk.transpose(3, 2, 0, 4, 1, 5))


def _w16_shard(w, core):
    """[H1, F] f32 -> [128, NFT3*H1] fp16: col kt*H1 + h = W[h, kt*128+p]*WS."""
    ws = w[:, core * FS:core * FS + NF3]
    wt = (ws.T * WS).astype(F16)                  # [4608, 256]
    return np.ascontiguousarray(
        wt.reshape(NFT3, 128, H1).transpose(1, 0, 2).reshape(128, NFT3 * H1))


def _w8_shard(w, core):
    """[H1, F] f32 -> [128, NFT8P, NHT, 2, 128] fp8e4 of W*WS, DR layout."""
    base = core * FS + NF3
    ws = w[:, base:base + NFT8P * 256]
    wt = (ws.T * WS).astype(E4)                   # [512, 256]
    wt = wt.reshape(NFT8P, 2, 128, NHT, 128)      # (j, i, p, ht, h')
    return np.ascontiguousarray(wt.transpose(2, 0, 3, 1, 4))


def kernel(white_features, black_features, W_fw, b_fw, W_fb, b_fb,
           W1, b1, W2, b2, W3, b3):
    white_features = np.asarray(white_features, dtype=F32)
    black_features = np.asarray(black_features, dtype=F32)
    W_fw = np.asarray(W_fw, dtype=F32)
    W_fb = np.asarray(W_fb, dtype=F32)

    # ---------- launch 1: feature transformer partials ----------
    nc1 = _get_compiled("ft", build_ft_kernel)
    in_maps1 = []
    for core in range(NCORES):
        feats3 = np.stack([_feat3_shard(white_features, core),
                           _feat3_shard(black_features, core)])
        feats8 = np.stack([_feat8_shard(white_features, core),
                           _feat8_shard(black_features, core)])
        wts16 = np.stack([_w16_shard(W_fw, core), _w16_shard(W_fb, core)])
        wts8 = np.stack([_w8_shard(W_fw, core), _w8_shard(W_fb, core)])
        in_maps1.append({"feats3": feats3, "feats8": feats8,
                         "wts16": wts16, "wts8": wts8})
    res1 = _run(nc1, in_maps1, "ft")
    partials = [np.asarray(r["partial"]) for r in res1.results]

    # ---------- host glue: reduce over F-shards + re-shard by batch ----
    total = np.zeros((2, NHT, 128, B), dtype=F32)
    for p in partials:
        total += p.astype(F32)

    # bias correction: every feature column is 0.5-shifted on device
    bc_w = (np.asarray(b_fw, np.float64)
            + 0.5 * W_fw.astype(np.float64).sum(1)).astype(F32)
    bc_b = (np.asarray(b_fb, np.float64)
            + 0.5 * W_fb.astype(np.float64).sum(1)).astype(F32)

    P = NXT * BSH
    consts = np.zeros((128, NCOL), dtype=F32)
    consts[:, 0:NXT * 32] = (
        np.asarray(W1, dtype=F32).T.reshape(NXT, 128, 32)
        .transpose(1, 0, 2).reshape(128, NXT * 32))
    consts[:, 128:128 + NHT] = bc_w.reshape(NHT, 128).T
    consts[:, 128 + NHT:128 + NXT] = bc_b.reshape(NHT, 128).T
    co = 128 + NXT
    consts[0:32, co:co + 32] = np.asarray(W2, dtype=F32).T
    consts[0:32, co + 32] = np.asarray(b1, dtype=F32)
    consts[0:32, co + 33] = np.asarray(b2, dtype=F32)
    consts[0:32, co + 34] = np.asarray(W3, dtype=F32).reshape(32)
    consts[0, co + 35] = np.asarray(b3, dtype=F32).reshape(())

    nc2 = _get_compiled("mlp", build_mlp_kernel)
    in_maps2 = []
    for core in range(NCORES):
        sl = total[..., core * BSH:(core + 1) * BSH]   # [2, NHT, 128, BSH]
        pre2 = np.empty((128, PREW), dtype=F32)
        pre2[:, 0:P] = sl.transpose(2, 0, 1, 3).reshape(128, P)
        pre2[:, P:] = consts
        in_maps2.append({"pre2": pre2})
    res2 = _run(nc2, in_maps2, "mlp")
    out = np.concatenate(
        [np.asarray(r["out"], dtype=F32).reshape(-1) for r in res2.results])
    return out
